# revision 19
# baseline (speedup 1.0000x reference)
# EpiGNN forward kernel for Trainium2 (Bass/Tile), data-parallel over batch:
# 8 batch elements -> 8 NeuronCores, no collectives.
#
# Key algorithmic restructurings vs the reference:
#  * RegionAwareConv = one matmul against a host-built Toeplitz-expanded
#    weight matrix (128=Fin*T contraction), maxpool over time done as a
#    free-dim tensor_reduce, branches padded to uniform T'=30 by duplicating
#    a valid column (max-invariant).
#  * Transmission-risk attention row sums use the Gram trick:
#    sum_j (q_i.k_j) = q_i.(sum_j k_j),  sum_j (q_i.k_j)^2 = q_i^T (k^T k) q_i
#    so the m x m attention matrix is never materialized.
#  * GraphLearner in the reference is dead code (adjacency_type='static') and
#    is skipped entirely.
#  * The row-normalized Laplacian is only ever compared with 0, and
#    lap[i,j] == 0 iff adj[i,j] == 0, so adj itself (cast bf16, transposed on
#    host) is the GAT mask.
#  * GAT softmax: exp(leakyrelu(el_i + er_j)) = max(A_i*B_j, A'_i*B'_j) with
#    A=exp(el), A'=exp(0.2*el), B=exp(er), B'=exp(0.2*er); attention is
#    computed unnormalized in tiles pT[j,q], the softmax denominator rides as
#    an extra all-ones column in the PV matmul lhsT.
import numpy as np
import ml_dtypes

import bass_rust
import concourse.bass as bass
import concourse.mybir as mybir
import concourse.tile as tile
from concourse.bass_utils import run_bass_kernel_spmd


def _split_waits(nc, maxw=1):
    # This container's walrus rejects instructions carrying more than one
    # sync wait; split extras into preceding single-wait NOPs.
    for f in nc.m.functions:
        for blk in f.blocks:
            out = []
            changed = False
            for inst in blk.instructions:
                si = inst.sync_info
                if si is not None and len(si.on_wait) > maxw:
                    waits = list(si.on_wait)
                    extra, keep = waits[:-maxw], waits[-maxw:]
                    for j in range(0, len(extra), maxw):
                        nop = mybir.InstNoOp(name=f"wsp_{inst.name}_{j}",
                                             ins=[], outs=[])
                        nop.engine = inst.engine
                        nop.sync_info = bass_rust.SyncInfo(
                            on_wait=extra[j:j + maxw], on_update=[])
                        out.append(nop)
                    inst.sync_info = bass_rust.SyncInfo(
                        on_wait=keep, on_update=list(si.on_update))
                    changed = True
                out.append(inst)
            if changed:
                il = blk.instructions
                il.clear()
                il.extend(out)

F32 = mybir.dt.float32
BF16 = mybir.dt.bfloat16
FP16 = mybir.dt.float16
AF = mybir.ActivationFunctionType
OP = mybir.AluOpType
AX = mybir.AxisListType

# problem dims (hardcoded per spec)
B, T, M, FIN = 8, 32, 1024, 4
KCH, HIDA, HIDR, H, FH, NL, TOUT = 8, 32, 40, 4, 10, 3, 12
NCORES = 8
LRELU = 0.2
BN_EPS = 1e-5
CONV_KS = [3, 5, 3, 5, 32]
CONV_DILS = [1, 1, 2, 2, 1]
TP = 30          # padded time length for pooled branches
NSL = M // 128   # 8 node slices
NCONV = 4 * KCH * TP + KCH  # 968 conv output columns

# tuning knobs
ACT_PATH_HEADS = ()          # heads (per layer) computed via Lrelu+Exp on ACT
MASK_ON_GPSIMD_ACT = True    # ACT-path mask multiply on gpsimd instead of DVE
MAX_ON_GPSIMD = ()           # DVE-path heads whose max() op goes to gpsimd


def _build(act_path_heads=ACT_PATH_HEADS, mask_on_gp=MASK_ON_GPSIMD_ACT,
           max_on_gp=MAX_ON_GPSIMD, debug_taps=False):
    nc = bass.Bass("TRN2", target_bir_lowering=False)
    taps = {}
    def tap(name, shape, dt=F32):
        if debug_taps:
            taps[name] = nc.dram_tensor("tap_" + name, shape, dt,
                                        kind="ExternalOutput").ap()
        return taps.get(name)

    # ---- I/O ----
    d_xconv = nc.dram_tensor("xconv", (128, M), F32, kind="ExternalInput").ap()
    d_adjT = nc.dram_tensor("adjT", (M, M), BF16, kind="ExternalInput").ap()
    d_wbig = nc.dram_tensor("wbig", (128, NCONV), F32, kind="ExternalInput").ap()
    d_bconv = nc.dram_tensor("bconv_bc", (128, HIDR), F32, kind="ExternalInput").ap()
    d_ident = nc.dram_tensor("ident", (128, 128), F32, kind="ExternalInput").ap()
    d_wq = nc.dram_tensor("wq", (HIDR, HIDA), F32, kind="ExternalInput").ap()
    d_wk = nc.dram_tensor("wk", (HIDR, HIDA), F32, kind="ExternalInput").ap()
    d_bqc = nc.dram_tensor("bq_col", (HIDA, 1), F32, kind="ExternalInput").ap()
    d_bqr = nc.dram_tensor("bq_row", (128, HIDA), F32, kind="ExternalInput").ap()
    d_bkr = nc.dram_tensor("bk_row", (128, HIDA), F32, kind="ExternalInput").ap()
    d_wtc = nc.dram_tensor("wt_col", (HIDR, 1), F32, kind="ExternalInput").ap()
    d_wsc = nc.dram_tensor("ws_col", (HIDR, 1), F32, kind="ExternalInput").ap()
    d_btsc = nc.dram_tensor("bts_col", (HIDR, 1), F32, kind="ExternalInput").ap()
    d_wgat = nc.dram_tensor("wgat", (NL, HIDR, HIDR), F32, kind="ExternalInput").ap()
    d_a1c = nc.dram_tensor("a1c", (NL, HIDR, H), F32, kind="ExternalInput").ap()
    d_a2c = nc.dram_tensor("a2c", (NL, HIDR, H), F32, kind="ExternalInput").ap()
    d_wout = nc.dram_tensor("wout", (4, HIDR, TOUT), F32, kind="ExternalInput").ap()
    d_boutc = nc.dram_tensor("bout_col", (TOUT, 1), F32, kind="ExternalInput").ap()
    d_res = nc.dram_tensor("res", (TOUT, M), F32, kind="ExternalOutput").ap()

    with tile.TileContext(nc) as tc:
        with (
            tc.tile_pool(name="consts", bufs=1) as cp,
            tc.tile_pool(name="persist", bufs=1) as pp,
            tc.tile_pool(name="work", bufs=3) as wp,
            tc.tile_pool(name="bcast", bufs=2) as bp,
            tc.tile_pool(name="ptiles", bufs=4) as ptp,
            tc.tile_pool(name="psA", bufs=2, space="PSUM") as psA,
            tc.tile_pool(name="dram", bufs=2, space="DRAM") as dp,
        ):
            # ---------- const loads ----------
            def cload(name, ap, dt=F32):
                t = cp.tile(list(ap.shape), dt, tag=name)
                nc.sync.dma_start(out=t, in_=ap)
                return t

            xconv = cload("xconv", d_xconv)
            wbig = cload("wbig", d_wbig)
            bconv = cload("bconv", d_bconv)
            ident = cload("ident", d_ident)
            wq = cload("wq", d_wq)
            wk = cload("wk", d_wk)
            bqc = cload("bqc", d_bqc)
            bqr = cload("bqr", d_bqr)
            bkr = cload("bkr", d_bkr)
            wtc = cload("wtc", d_wtc)
            wsc = cload("wsc", d_wsc)
            btsc = cload("btsc", d_btsc)
            wgat = cp.tile([HIDR, NL, HIDR], F32, tag="wgat")
            a1c = cp.tile([HIDR, NL, H], F32, tag="a1c")
            a2c = cp.tile([HIDR, NL, H], F32, tag="a2c")
            wout = cp.tile([HIDR, 4, TOUT], F32, tag="wout")
            for l in range(NL):
                nc.sync.dma_start(out=wgat[:, l, :], in_=d_wgat[l])
                nc.sync.dma_start(out=a1c[:, l, :], in_=d_a1c[l])
                nc.sync.dma_start(out=a2c[:, l, :], in_=d_a2c[l])
            for ci in range(4):
                nc.sync.dma_start(out=wout[:, ci, :], in_=d_wout[ci])
            boutc = cload("boutc", d_boutc)

            maskT = cp.tile([128, NSL, M], BF16, tag="maskT")
            for jt in range(NSL):
                nc.sync.dma_start(out=maskT[:, jt, :],
                                  in_=d_adjT[jt * 128:(jt + 1) * 128, :])

            ones40 = cp.tile([128, HIDR], BF16, tag="ones40")
            nc.vector.memset(ones40, 1.0)

            # persistent activations
            tembT = pp.tile([HIDR, M], F32, tag="tembT")      # temp_emb^T
            g1T = pp.tile([HIDR, M], F32, tag="g1T")
            g2T = pp.tile([HIDR, M], F32, tag="g2T")
            g3T = pp.tile([HIDR, M], F32, tag="g3T")
            featT = pp.tile([HIDR, M], F32, tag="featT")
            whT = pp.tile([HIDR, M], F32, tag="whT")          # Wh^T (per layer)
            whones = pp.tile([128, NSL, H * (FH + 1)], FP16, tag="whones")
            qT = pp.tile([HIDA, M], F32, tag="qT")
            kext = pp.tile([128, NSL, HIDA + 1], F32, tag="kext")
            qm = pp.tile([128, NSL, HIDA], F32, tag="qm")
            k2e = pp.tile([HIDA, HIDA + 1], F32, tag="k2e")
            s1c = pp.tile([128, NSL], F32, tag="s1c")
            s2c = pp.tile([128, NSL], F32, tag="s2c")
            tinT = pp.tile([NSL, 128], F32, tag="tinT")
            tin_bc = pp.tile([HIDR, M], F32, tag="tin_bc")
            Bn = pp.tile([128, NSL, H], F32, tag="Bn")        # exp(er-E)
            B2n = pp.tile([128, NSL, H], F32, tag="B2n")      # exp(.2(er-E))
            arow = pp.tile([H, M], BF16, tag="arow")          # exp(min(.8m,0))
            a2row = pp.tile([H, M], BF16, tag="a2row")        # exp(min(-.8m,0))
            negE = pp.tile([128, H], F32, tag="negE")
            neg02E = pp.tile([128, H], F32, tag="neg02E")
            Ecol4 = pp.tile([H, 1], F32, tag="Ecol4")
            epos = pp.tile([128, H], F32, tag="epos")
            m4 = pp.tile([H, M], F32, tag="m4")
            hp = pp.tile([HIDR, M], F32, tag="hp")            # pre-elu gat out
            res_sb = pp.tile([TOUT, M], F32, tag="res_sb")

            # ---------- conv backbone ----------
            for s in range(NSL):
                y = psA.tile([128, NCONV], F32, tag="mmA")
                xs = xconv[:, s * 128:(s + 1) * 128]
                nc.tensor.matmul(y[:, 0:512], xs, wbig[:, 0:512],
                                 start=True, stop=True)
                nc.tensor.matmul(y[:, 512:NCONV], xs, wbig[:, 512:NCONV],
                                 start=True, stop=True)
                te = wp.tile([128, HIDR], F32, tag="te")
                nc.vector.tensor_reduce(
                    out=te[:, 0:32],
                    in_=y[:, 0:960].rearrange("p (g t) -> p g t", t=TP),
                    axis=AX.X, op=OP.max)
                nc.scalar.copy(te[:, 32:40], y[:, 960:968])
                nc.vector.tensor_tensor(te, te, bconv, op=OP.add)
                nc.scalar.activation(te, te, AF.Tanh)
                nc.scalar.activation(te, te, AF.Tanh)
                tr = psA.tile([HIDR, 128], F32, tag="mmA")
                nc.tensor.transpose(tr, te, ident)
                nc.scalar.copy(tembT[:, s * 128:(s + 1) * 128], tr)

            if debug_taps:
                nc.sync.dma_start(out=tap("tembT", (HIDR, M)), in_=tembT)
            # ---------- transmission-risk attention (Gram trick) ----------
            qps = psA.tile([HIDA, M], F32, tag="mmA")
            nc.tensor.matmul(qps[:, 0:512], wq, tembT[:, 0:512], start=True, stop=True)
            nc.tensor.matmul(qps[:, 512:M], wq, tembT[:, 512:M], start=True, stop=True)
            nc.scalar.activation(qT, qps, AF.Identity, bias=bqc)

            nc.vector.memset(kext, 1.0)
            for s in range(NSL):
                sl = slice(s * 128, (s + 1) * 128)
                kps = psA.tile([128, HIDA], F32, tag="mmA")
                nc.tensor.matmul(kps, tembT[:, sl], wk, start=True, stop=True)
                nc.vector.tensor_tensor(kext[:, s, 0:HIDA], kps, bkr, op=OP.add)
                qmp = psA.tile([128, HIDA], F32, tag="mmA")
                nc.tensor.matmul(qmp, tembT[:, sl], wq, start=True, stop=True)
                nc.vector.tensor_tensor(qm[:, s, :], qmp, bqr, op=OP.add)
            k2p = psA.tile([HIDA, HIDA + 1], F32, tag="mmA")
            for s in range(NSL):
                nc.tensor.matmul(k2p, kext[:, s, 0:HIDA], kext[:, s, :],
                                 start=(s == 0), stop=(s == NSL - 1))
            nc.scalar.copy(k2e, k2p)
            for s in range(NSL):
                tqp = psA.tile([128, HIDA + 1], F32, tag="mmA")
                nc.tensor.matmul(tqp, qT[:, s * 128:(s + 1) * 128], k2e,
                                 start=True, stop=True)
                scr = wp.tile([128, HIDA], F32, tag="scr")
                nc.vector.scalar_tensor_tensor(
                    out=scr, in0=tqp[:, 0:HIDA], scalar=1.0, in1=qm[:, s, :],
                    op0=OP.mult, op1=OP.mult, accum_out=s2c[:, s:s + 1])
                nc.scalar.copy(s1c[:, s:s + 1], tqp[:, HIDA:HIDA + 1])
            rt = wp.tile([128, NSL], F32, tag="rt")
            nc.scalar.activation(rt, s2c, AF.Sqrt)
            nc.vector.tensor_scalar(out=rt, in0=rt, scalar1=1e-12, scalar2=None,
                                    op0=OP.max)
            nc.vector.reciprocal(rt, rt)
            nc.vector.tensor_tensor(rt, rt, s1c, op=OP.mult)  # rt = t_in cols
            trp = psA.tile([NSL, 128], F32, tag="mmA")
            nc.tensor.transpose(trp, rt[:, 0:NSL], ident)
            nc.scalar.copy(tinT, trp)
            dr_tin = dp.tile([NSL, 128], F32, tag="dr_tin")
            nc.sync.dma_start(out=dr_tin, in_=tinT)
            for s in range(NSL):
                nc.gpsimd.dma_start(
                    out=tin_bc[:, s * 128:(s + 1) * 128],
                    in_=dr_tin[s:s + 1, :].to_broadcast((HIDR, 128)))

            # ---------- degree row (broadcast) + feat_emb ----------
            dbc = psA.tile([HIDR, M], F32, tag="pv")
            for jt in range(NSL):
                for nh in range(2):
                    nc.tensor.matmul(
                        dbc[:, nh * 512:(nh + 1) * 512], ones40,
                        maskT[:, jt, nh * 512:(nh + 1) * 512],
                        start=(jt == 0), stop=(jt == NSL - 1))
            f1 = wp.tile([HIDR, M], F32, tag="f1")
            nc.vector.scalar_tensor_tensor(out=f1, in0=tin_bc, scalar=wtc,
                                           in1=tembT, op0=OP.mult, op1=OP.add)
            nc.vector.scalar_tensor_tensor(out=f1, in0=dbc, scalar=wsc,
                                           in1=f1, op0=OP.mult, op1=OP.add)
            nc.scalar.activation(featT, f1, AF.Identity, bias=btsc)
            if debug_taps:
                nc.sync.dma_start(out=tap("featT", (HIDR, M)), in_=featT)
                nc.sync.dma_start(out=tap("tin_bc", (HIDR, M)), in_=tin_bc)

            # ---------- GAT layers ----------
            layer_in = [featT, g1T, g2T]
            layer_out = [g1T, g2T, g3T]
            for l in range(NL):
                hT = layer_in[l]
                # Wh^T = wgat[l]^T applied in T-layout
                whp = psA.tile([HIDR, M], F32, tag="mmA")
                nc.tensor.matmul(whp[:, 0:512], wgat[:, l, :], hT[:, 0:512],
                                 start=True, stop=True)
                nc.tensor.matmul(whp[:, 512:M], wgat[:, l, :], hT[:, 512:M],
                                 start=True, stop=True)
                nc.scalar.copy(whT, whp)
                # Wh in node-slice layout (+ ones col) for PV lhsT
                nc.vector.memset(whones, 1.0)
                for s in range(NSL):
                    wnp = psA.tile([128, HIDR], F32, tag="mmA")
                    nc.tensor.matmul(wnp, hT[:, s * 128:(s + 1) * 128],
                                     wgat[:, l, :], start=True, stop=True)
                    dst = whones[:, s, :].rearrange("p (h k) -> p h k",
                                                    k=FH + 1)[:, :, 0:FH]
                    nc.scalar.copy(dst, wnp.rearrange("p (h f) -> p h f", f=FH))
                # er in node-slice layout (needed first: E = max_j er)
                erp = psA.tile([128, NSL * H], F32, tag="mmA")
                erp3 = erp.rearrange("p (s h) -> p s h", h=H)
                for s in range(NSL):
                    nc.tensor.matmul(erp3[:, s, :],
                                     whT[:, s * 128:(s + 1) * 128],
                                     a2c[:, l, :], start=True, stop=True)
                # E_h = max over all nodes of er, via an er row-layout matmul
                # and a free-dim reduce (exps are shifted by E for stability).
                erow = psA.tile([H, M], F32, tag="mmA")
                nc.tensor.matmul(erow[:, 0:512], a2c[:, l, :], whT[:, 0:512],
                                 start=True, stop=True)
                nc.tensor.matmul(erow[:, 512:M], a2c[:, l, :], whT[:, 512:M],
                                 start=True, stop=True)
                nc.vector.tensor_reduce(out=Ecol4, in_=erow, axis=AX.X,
                                        op=OP.max)
                dr_e = dp.tile([H, 1], F32, tag="dr_e")
                nc.sync.dma_start(out=dr_e, in_=Ecol4)
                nc.gpsimd.dma_start(
                    out=epos,
                    in_=dr_e.rearrange("h o -> o h").to_broadcast((128, H)))
                nc.vector.tensor_scalar(out=negE, in0=epos, scalar1=-1.0,
                                        scalar2=None, op0=OP.mult)
                nc.vector.tensor_scalar(out=neg02E, in0=epos, scalar1=-LRELU,
                                        scalar2=None, op0=OP.mult)
                for h in range(H):
                    nc.scalar.activation(Bn[:, :, h], erp3[:, :, h], AF.Exp,
                                         bias=negE[:, h:h + 1])
                    nc.scalar.activation(B2n[:, :, h], erp3[:, :, h], AF.Exp,
                                         bias=neg02E[:, h:h + 1], scale=LRELU)
                # el rows; m = el + E, c = lrelu(m) folded into A/A'
                elp = psA.tile([H, M], F32, tag="mmA")
                nc.tensor.matmul(elp[:, 0:512], a1c[:, l, :], whT[:, 0:512],
                                 start=True, stop=True)
                nc.tensor.matmul(elp[:, 512:M], a1c[:, l, :], whT[:, 512:M],
                                 start=True, stop=True)
                nc.scalar.activation(m4, elp, AF.Identity, bias=Ecol4)
                tt1 = wp.tile([H, M], F32, tag="tt1")
                nc.vector.tensor_scalar(out=tt1, in0=m4, scalar1=0.8,
                                        scalar2=0.0, op0=OP.mult, op1=OP.min)
                nc.scalar.activation(arow, tt1, AF.Exp)
                nc.vector.tensor_scalar(out=tt1, in0=m4, scalar1=-0.8,
                                        scalar2=0.0, op0=OP.mult, op1=OP.min)
                nc.scalar.activation(a2row, tt1, AF.Exp)
                dr_a = dp.tile([H, M], BF16, tag="dr_a")
                dr_a2 = dp.tile([H, M], BF16, tag="dr_a2")
                nc.sync.dma_start(out=dr_a, in_=arow)
                nc.sync.dma_start(out=dr_a2, in_=a2row)

                for h in range(H):
                    abc = bp.tile([128, M], BF16, tag="abc")
                    nc.gpsimd.dma_start(out=abc,
                                        in_=dr_a[h:h + 1, :].to_broadcast((128, M)))
                    a2bc = bp.tile([128, M], BF16, tag="a2bc")
                    nc.gpsimd.dma_start(out=a2bc,
                                        in_=dr_a2[h:h + 1, :].to_broadcast((128, M)))
                    pvp = psA.tile([FH + 1, M], F32, tag="pv")
                    for jt in range(NSL):
                        p = ptp.tile([128, M], BF16, tag="p")
                        u = wp.tile([128, M], BF16, tag="u")
                        nc.vector.scalar_tensor_tensor(
                            out=u, in0=abc, scalar=Bn[:, jt, h:h + 1],
                            in1=maskT[:, jt, :], op0=OP.mult, op1=OP.mult)
                        v = wp.tile([128, M], BF16, tag="v")
                        nc.vector.scalar_tensor_tensor(
                            out=v, in0=a2bc, scalar=B2n[:, jt, h:h + 1],
                            in1=maskT[:, jt, :], op0=OP.mult, op1=OP.mult)
                        eng = nc.gpsimd if (h in max_on_gp) else nc.vector
                        eng.tensor_max(p, u, v)
                        lhsT = whones[:, jt, h * (FH + 1):(h + 1) * (FH + 1)]
                        nc.tensor.matmul(pvp[:, 0:512], lhsT, p[:, 0:512],
                                         start=(jt == 0), stop=(jt == NSL - 1))
                        nc.tensor.matmul(pvp[:, 512:M], lhsT, p[:, 512:M],
                                         start=(jt == 0), stop=(jt == NSL - 1))
                    # divide by denominator (row FH) and park in hp
                    rall = wp.tile([FH + 1, M], F32, tag="rall")
                    nc.vector.reciprocal(rall, pvp)
                    dr_r = dp.tile([1, M], F32, tag="dr_r")
                    nc.sync.dma_start(out=dr_r, in_=rall[FH:FH + 1, :])
                    rbc = wp.tile([FH, M], F32, tag="rbc")
                    nc.gpsimd.dma_start(out=rbc,
                                        in_=dr_r[0:1, :].to_broadcast((FH, M)))
                    hp_h = wp.tile([FH, M], F32, tag="hp_h")
                    nc.vector.tensor_tensor(hp_h, pvp[0:FH, :], rbc, op=OP.mult)
                    # assemble into the 40-row hp tile (DMA: engines cannot
                    # write at non-32-aligned partition offsets)
                    nc.sync.dma_start(out=hp[h * FH:(h + 1) * FH, :], in_=hp_h)
                if debug_taps and l == 0:
                    nc.sync.dma_start(out=tap("whT0", (HIDR, M)), in_=whT)
                    nc.sync.dma_start(out=tap("arow0", (H, M), BF16), in_=arow)
                    nc.sync.dma_start(out=tap("Bn0", (128, NSL * H)),
                                      in_=Bn.rearrange("p s h -> p (s h)"))
                    nc.sync.dma_start(out=tap("hp0", (HIDR, M)), in_=hp)
                # elu
                tmin = wp.tile([HIDR, M], F32, tag="tmin")
                nc.vector.tensor_scalar(out=tmin, in0=hp, scalar1=0.0,
                                        scalar2=None, op0=OP.min)
                nc.scalar.activation(tmin, tmin, AF.Exp)
                nc.vector.scalar_tensor_tensor(out=layer_out[l], in0=tmin,
                                               scalar=-1.0, in1=hp,
                                               op0=OP.add, op1=OP.max)

            if debug_taps:
                nc.sync.dma_start(out=tap("g1T", (HIDR, M)), in_=g1T)
                nc.sync.dma_start(out=tap("g3T", (HIDR, M)), in_=g3T)
            # ---------- output projection ----------
            rps = psA.tile([TOUT, M], F32, tag="mmA")
            srcs = [g1T, g2T, g3T, featT]
            for nh in range(2):
                sl = slice(nh * 512, (nh + 1) * 512)
                for ci, src in enumerate(srcs):
                    nc.tensor.matmul(rps[:, sl], wout[:, ci, :], src[:, sl],
                                     start=(ci == 0), stop=(ci == 3))
            nc.scalar.activation(res_sb, rps, AF.Identity, bias=boutc)
            nc.sync.dma_start(out=d_res, in_=res_sb)

    return nc


def _host_prep(X, adj, conv_params, WQ, bQ, WK, bK, Wt, bt, Ws, bs_,
               Wg1, bg1, Wg2, bg2, gat_params, Wout, bout):
    f32 = np.float32
    X = np.asarray(X, f32)
    adj = np.asarray(adj, f32)
    inv = 1.0 / np.sqrt(1.0 + BN_EPS)

    wbig = np.zeros((128, NCONV), f32)
    bconv = np.zeros((HIDR,), f32)
    for br, (params, ks, dil) in enumerate(zip(conv_params, CONV_KS, CONV_DILS)):
        w, bb, gamma, beta = [np.asarray(p, f32) for p in params]
        scale = inv * gamma
        bconv[br * KCH:(br + 1) * KCH] = bb * scale + beta
        tlen = T - dil * (ks - 1)
        if br < 4:
            for o in range(KCH):
                for tp in range(TP):
                    tsrc = min(tp, tlen - 1)  # duplicate last valid col (max-safe)
                    col = br * KCH * TP + o * TP + tp
                    for i in range(FIN):
                        for s in range(ks):
                            wbig[i * T + tsrc + dil * s, col] += \
                                w[o, i, s, 0] * scale[o]
        else:
            for o in range(KCH):
                col = 4 * KCH * TP + o
                for i in range(FIN):
                    for s in range(ks):
                        wbig[i * T + dil * s, col] += w[o, i, s, 0] * scale[o]

    def col(x):
        return np.ascontiguousarray(np.asarray(x, f32).reshape(-1, 1))

    def bcast(x, p=128):
        return np.ascontiguousarray(
            np.broadcast_to(np.asarray(x, f32).reshape(1, -1), (p, len(np.asarray(x).ravel()))))

    wgat = np.stack([np.asarray(w, f32) for w, _ in gat_params])
    a1c = np.zeros((NL, HIDR, H), f32)
    a2c = np.zeros((NL, HIDR, H), f32)
    for l, (_, a) in enumerate(gat_params):
        a = np.asarray(a, f32)
        for h in range(H):
            a1c[l, h * FH:(h + 1) * FH, h] = a[h, :FH]
            a2c[l, h * FH:(h + 1) * FH, h] = a[h, FH:]

    Wout = np.asarray(Wout, f32)
    weights = dict(
        wbig=wbig,
        bconv_bc=bcast(bconv),
        ident=np.eye(128, dtype=f32),
        wq=np.asarray(WQ, f32), wk=np.asarray(WK, f32),
        bq_col=col(bQ), bq_row=bcast(bQ), bk_row=bcast(bK),
        wt_col=col(np.asarray(Wt, f32)[0]), ws_col=col(np.asarray(Ws, f32)[0]),
        bts_col=col(np.asarray(bt, f32) + np.asarray(bs_, f32)),
        wgat=wgat, a1c=a1c, a2c=a2c,
        wout=np.ascontiguousarray(Wout.reshape(4, HIDR, TOUT)),
        bout_col=col(bout),
    )

    per_core = []
    for b in range(B):
        xc = np.ascontiguousarray(
            X[b].transpose(2, 0, 1).reshape(128, M))  # (Fin*T, m)
        adjT = np.ascontiguousarray(adj[b].T).astype(ml_dtypes.bfloat16)
        m = dict(weights)
        m["xconv"] = xc
        m["adjT"] = adjT
        per_core.append(m)
    return per_core


_NC_CACHE = {}


def kernel(**inputs):
    key = (ACT_PATH_HEADS, MASK_ON_GPSIMD_ACT, MAX_ON_GPSIMD)
    if key not in _NC_CACHE:
        nc = _build()
        _split_waits(nc)
        _NC_CACHE[key] = nc
    nc = _NC_CACHE[key]
    in_maps = _host_prep(**inputs)
    last = None
    for _ in range(3):
        try:
            r = run_bass_kernel_spmd(nc, in_maps, core_ids=list(range(NCORES)))
            out = np.stack([r.results[c]["res"] for c in range(NCORES)])
            return out.astype(np.float32)
        except Exception as e:  # transient device wedges resolve on retry
            last = e
    raise last
